# revision 2
# baseline (speedup 1.0000x reference)
"""GQA attention kernel for 8 TRN2 NeuronCores.

Problem: B=2, T=2048, C=4096, NH=32 q-heads, NKV=8 kv-heads, HD=128,
RoPE (theta=1e4), causal, f32 I/O.

Sharding: core = (batch b, kv-head-group g): b = core//4, g = core%4.
Each core owns batch b and kv heads {2g, 2g+1} (= q heads 8g..8g+7):
  - projects x[b] against its wq/wk/wv column slices (bf16 compute),
  - runs causal attention for its 8 q heads,
  - computes the partial o_proj x its wo row slice -> [T, C] f32.
Host sums the 4 partials per batch.

On-chip layout is feature-major ("X^T"): activations live as
[feature=partition, token=free] so every matmul contracts along
partitions. x is pre-transposed/bf16-cast on host; RoPE's rotate_half
is a 128x128 permutation matmul on the PE.

Attention uses a "flipped" PV: the P^T tiles produced by the score
matmuls stream through the PE against a stationary V-natural tile,
accumulating O^T (feature-major) directly in PSUM -- this streams 512
columns per weight load (vs 129 in the natural orientation, which is
LDWEIGHTS-bound) and removes the post-attention O transposes entirely.
The softmax denominator is recovered by a [128,1]-ones matmul over a
DVE-accumulated column-sum tile, broadcast back over partitions with a
[1,128]-ones matmul, and folded into the PSUM->SBUF normalization
multiply.
"""

import sys

sys.path.insert(0, "/opt/trn_rl_repo")

import numpy as np
import ml_dtypes

import concourse.bass as bass
import concourse.bacc as bacc
import concourse.mybir as mybir
import concourse.tile as tile
from concourse.bass_utils import run_bass_kernel_spmd

BF16 = mybir.dt.bfloat16
F32 = mybir.dt.float32
AF = mybir.ActivationFunctionType
ALU = mybir.AluOpType

B, T, C = 2, 2048, 4096
NH, NKV, HD = 32, 8, 128
THETA = 10000.0
NCORES = 8

QH = 8          # q heads per core
KV = 2          # kv heads per core
OUTS = 12       # projection output tiles: 8 q + 2 k + 2 v
QC = 4          # token chunks of 512
KT = 16         # k tiles of 128
TT = 16         # token tiles of 128
CCH = 32        # contraction chunks of 128 over C

_CACHE = {}


def _build_nc():
    nc = bacc.Bacc("TRN2", target_bir_lowering=False, debug=False,
                   enable_asserts=False, num_devices=NCORES)

    xT_d = nc.dram_tensor("xT", [C, T], BF16, kind="ExternalInput")
    wqkv_d = nc.dram_tensor("wqkv", [CCH, 2, 128, 768], BF16, kind="ExternalInput")
    wo_d = nc.dram_tensor("wo", [QH * HD, C], BF16, kind="ExternalInput")
    cos_d = nc.dram_tensor("cosT", [128, T], BF16, kind="ExternalInput")
    sin_d = nc.dram_tensor("sinT", [128, T], BF16, kind="ExternalInput")
    prot_d = nc.dram_tensor("protT", [128, 128], BF16, kind="ExternalInput")
    ident_d = nc.dram_tensor("ident", [128, 128], BF16, kind="ExternalInput")
    cmask_d = nc.dram_tensor("cmask", [128, 4, 512], F32, kind="ExternalInput")
    out_d = nc.dram_tensor("out", [T, C], F32, kind="ExternalOutput")

    with tile.TileContext(nc) as tc:
        with tc.tile_pool(name="persist", bufs=1) as pp:
            ident = pp.tile([128, 128], BF16)
            nc.sync.dma_start(ident, ident_d.ap())
            cosT = pp.tile([128, T], BF16)
            sinT = pp.tile([128, T], BF16)
            prot = pp.tile([128, 128], BF16)
            cmask = pp.tile([128, 4, 512], F32)
            ones_col = pp.tile([128, 1], BF16)
            ones_row = pp.tile([1, 128], BF16)
            warm = pp.tile([128, 128], BF16)
            nc.vector.memset(ones_col, 1.0)
            nc.vector.memset(ones_row, 1.0)
            nc.vector.memset(warm, 0.25)

            # HAM warm-up with REAL matmuls (transposes don't count as
            # PE-busy for the clock gate). No DMA dependency: operand is
            # memset on-chip, so this starts immediately and spans the
            # ~8us until the first x^T block lands, leaving the PE at
            # 2.4 GHz when projections begin.
            with tc.tile_pool(name="pwarm", bufs=2, space="PSUM") as pwp:
                for w in range(96):
                    wps = pwp.tile([128, 128], F32, name=f"warm{w}", tag="warm")
                    nc.tensor.matmul(wps, warm, warm, start=True, stop=True)

            QT = pp.tile([128, QH, T], BF16)
            KTt = pp.tile([128, KV, T], BF16)
            VT = pp.tile([128, KV, T], BF16)
            OT = pp.tile([128, QH, T], BF16)
            Vn = pp.tile([128, KV, KT, 128], BF16)

            # ---------------- projections: Q^T/K^T/V^T = W^T @ x^T ----------
            with tc.tile_pool(name="xt", bufs=2) as xtp, \
                 tc.tile_pool(name="wt", bufs=6) as wtp, \
                 tc.tile_pool(name="pproj", bufs=7, space="PSUM") as ppj:
                xview = xT_d.ap().rearrange("(c p) t -> p c t", p=128)
                for qc in range(QC):
                    tsl = slice(qc * 512, (qc + 1) * 512)
                    xt = xtp.tile([128, CCH, 512], BF16)
                    # split the load (early c-chunks land first) and use the
                    # scalar HWDGE queue so weights stream in parallel on sync
                    for piece in range(4):
                        csl = slice(piece * 8, (piece + 1) * 8)
                        nc.scalar.dma_start(xt[:, csl, :], xview[:, csl, tsl])
                    for grp in range(2):
                        psums = [ppj.tile([128, 512], F32, name=f"pj{qc}_{grp}_{o}",
                                          tag="pj") for o in range(6)]
                        for c in range(CCH):
                            wt = wtp.tile([128, 768], BF16)
                            nc.sync.dma_start(wt, wqkv_d.ap()[c, grp])
                            for o in range(6):
                                nc.tensor.matmul(
                                    psums[o], wt[:, o * 128:(o + 1) * 128],
                                    xt[:, c, :], start=(c == 0), stop=(c == CCH - 1))
                        for o in range(6):
                            oi = grp * 6 + o
                            if oi < 8:
                                dst = QT[:, oi, tsl]
                            elif oi < 10:
                                dst = KTt[:, oi - 8, tsl]
                            else:
                                dst = VT[:, oi - 10, tsl]
                            # alternate engines so psum slots free faster
                            if o % 2 == 0:
                                nc.scalar.copy(dst, psums[o])
                            else:
                                nc.vector.tensor_copy(dst, psums[o])

            # constants for RoPE/attention — loaded once projections are
            # underway so they don't delay the first weight tiles
            nc.scalar.dma_start(cosT, cos_d.ap())
            nc.scalar.dma_start(sinT, sin_d.ap())
            nc.scalar.dma_start(prot, prot_d.ap())
            nc.scalar.dma_start(cmask, cmask_d.ap())

            # wo load after the x^T/weight stream pools are gone, so it
            # overlaps RoPE + attention without blowing SBUF
            wo_pool = tc.alloc_tile_pool(name="wop", bufs=1)
            wo_t = wo_pool.tile([128, QH, C], BF16)
            nc.sync.dma_start(wo_t, wo_d.ap().rearrange("(h p) n -> p h n", p=128))

            # ---------------- attention (with fused RoPE) ------------------
            # rot = P_rot @ q (sign baked into P_rot), q' = q*cos + rot*sin
            # S^T[k,q] = K @ Q^T; P^T = exp(S^T + mask)
            # O^T[hd,q] = sum_kt V_nat[kt]^T @ P^T[kt]   (flipped PV)
            # den[q] = ones^T @ (sum_kt P^T[kt]); O = O^T * (1/den) bcast
            with tc.tile_pool(name="pst", bufs=3, space="PSUM") as pst, \
                 tc.tile_pool(name="pot", bufs=2, space="PSUM") as pot, \
                 tc.tile_pool(name="pdb", bufs=3, space="PSUM") as pdb, \
                 tc.tile_pool(name="pt", bufs=6) as ptp, \
                 tc.tile_pool(name="acc", bufs=2) as accp, \
                 tc.tile_pool(name="rcd", bufs=2) as rcdp, \
                 tc.tile_pool(name="bs", bufs=2) as bsp, \
                 tc.tile_pool(name="ropes", bufs=3) as rsp:

                def rope(src):
                    for rqc in range(QC):
                        rsl = slice(rqc * 512, (rqc + 1) * 512)
                        ps = pdb.tile([128, 512], F32, name=f"rot{rqc}", tag="db")
                        nc.tensor.matmul(ps, prot, src[:, rsl], start=True,
                                         stop=True)
                        rs = rsp.tile([128, 512], BF16, name=f"rs{rqc}", tag="rs")
                        nc.vector.tensor_tensor(rs, ps, sinT[:, rsl], op=ALU.mult)
                        nc.vector.tensor_tensor(src[:, rsl], src[:, rsl],
                                                cosT[:, rsl], op=ALU.mult)
                        nc.vector.tensor_tensor(src[:, rsl], src[:, rsl], rs,
                                                op=ALU.add)

                def vtrans(kv):
                    for kt in range(KT):
                        pt = pst.tile([128, 128], BF16, name=f"tv{kv}_{kt}",
                                      tag="st")
                        nc.tensor.transpose(
                            pt, VT[:, kv, kt * 128:(kt + 1) * 128], ident)
                        nc.vector.tensor_copy(Vn[:, kv, kt, :], pt)

                rope(KTt[:, 0, :])
                vtrans(0)
                rope(QT[:, 0, :])
                rope(KTt[:, 1, :])
                vtrans(1)

                # pending = (otps, rcd_tile, h, tsl) of the previous chunk;
                # its broadcast + normalize are emitted inside the next
                # chunk's kt loop so the DVE reciprocal is never on the PE
                # critical path.
                pending = [None]

                def flush_pending():
                    if pending[0] is None:
                        return
                    p_ot, p_rcd, p_h, p_tsl = pending[0]
                    pending[0] = None
                    Bp = pdb.tile([128, 512], F32, tag="db")
                    nc.tensor.matmul(Bp, ones_row, p_rcd, start=True, stop=True)
                    Bs = bsp.tile([128, 512], BF16)
                    nc.vector.tensor_copy(Bs, Bp)
                    nc.vector.tensor_tensor(OT[:, p_h, p_tsl], p_ot, Bs,
                                            op=ALU.mult)

                for h in range(QH):
                    kv = h // 4
                    if h + 1 < QH:
                        rope(QT[:, h + 1, :])
                    for qc in range(QC):
                        tsl = slice(qc * 512, (qc + 1) * 512)
                        last_kt = 4 * qc + 3
                        otps = pot.tile([128, 512], F32, name=f"ot{h}_{qc}",
                                        tag="ot")
                        acc = accp.tile([128, 512], BF16)
                        last_pt = None
                        for kt in range(4 * qc + 4):
                            d = kt - 4 * qc
                            st = pst.tile([128, 512], F32, tag="st")
                            ptile = ptp.tile([128, 512], BF16)
                            if d >= 0:
                                # columns < d*128 are fully masked: skip them
                                # in the score stream, exp, PV and acc; only
                                # the [d*128,(d+1)*128) block straddles the
                                # diagonal and needs the additive mask
                                vsl = slice(d * 128, 512)
                                bsl = slice(d * 128, (d + 1) * 128)
                                nc.tensor.matmul(
                                    st[:, vsl],
                                    KTt[:, kv, kt * 128:(kt + 1) * 128],
                                    QT[:, h, qc * 512 + d * 128:(qc + 1) * 512],
                                    start=True, stop=True)
                                nc.vector.tensor_tensor(
                                    st[:, bsl], st[:, bsl], cmask[:, d, bsl],
                                    op=ALU.add)
                                nc.scalar.activation(ptile[:, vsl], st[:, vsl],
                                                     AF.Exp)
                            else:
                                vsl = slice(0, 512)
                                nc.tensor.matmul(
                                    st, KTt[:, kv, kt * 128:(kt + 1) * 128],
                                    QT[:, h, tsl], start=True, stop=True)
                                nc.scalar.activation(ptile, st, AF.Exp)
                            nc.tensor.matmul(
                                otps[:, vsl], Vn[:, kv, kt, :], ptile[:, vsl],
                                start=(kt == 0), stop=(kt == last_kt))
                            # column-sum accumulator for the softmax
                            # denominator; the last tile is folded into the
                            # den matmul directly (depends only on ACT, not
                            # the DVE queue)
                            if kt == 0:
                                nc.vector.tensor_copy(acc, ptile)
                            elif kt < last_kt:
                                nc.vector.tensor_tensor(
                                    acc[:, vsl], acc[:, vsl], ptile[:, vsl],
                                    op=ALU.add)
                            else:
                                last_pt = ptile
                            if kt == 2:
                                flush_pending()
                        den = pdb.tile([128, 512], F32, tag="db")
                        nc.tensor.matmul(den[0:1, :], ones_col, acc,
                                         start=True, stop=False)
                        nc.tensor.matmul(den[0:1, 384:512], ones_col,
                                         last_pt[:, 384:512],
                                         start=False, stop=True)
                        rcd = rcdp.tile([1, 512], BF16)
                        with nc.allow_low_precision("softmax denom in bf16"):
                            nc.vector.reciprocal(rcd, den[0:1, :])
                        pending[0] = (otps, rcd, h, tsl)
                flush_pending()

            # ---------------- o_proj partial: O @ wo_slice ----------------
            with tc.tile_pool(name="pout", bufs=6, space="PSUM") as outp, \
                 tc.tile_pool(name="ostg", bufs=6) as stgp:
                for tt in range(TT):
                    psl = slice(tt * 128, (tt + 1) * 128)
                    for n in range(8):
                        nsl = slice(n * 512, (n + 1) * 512)
                        ps = outp.tile([128, 512], F32)
                        for h in range(QH):
                            nc.tensor.matmul(ps, OT[:, h, psl],
                                             wo_t[:, h, nsl],
                                             start=(h == 0), stop=(h == QH - 1))
                        stg = stgp.tile([128, 512], F32)
                        nc.scalar.copy(stg, ps)
                        nc.sync.dma_start(out_d.ap()[psl, nsl], stg)

            wo_pool.release()

    nc.compile()
    return nc


def _host_prep(x, wq, wk, wv, wo):
    bf = ml_dtypes.bfloat16
    scale = HD ** -0.5

    # RoPE tables, feature-major [128, T]
    inv_freq = 1.0 / (THETA ** (np.arange(0, HD, 2, dtype=np.float32) / HD))
    t = np.arange(T, dtype=np.float32)
    freqs = np.outer(t, inv_freq)                      # [T, 64]
    emb = np.concatenate([freqs, freqs], -1)           # [T, 128]
    cosT = np.ascontiguousarray(np.cos(emb).T).astype(bf)
    sinT = np.ascontiguousarray(np.sin(emb).T).astype(bf)

    # rotate_half as a permutation matrix, pre-transposed for lhsT:
    # rot = P_rot @ q with P_rot[i, i+64] = -1 (i<64), P_rot[i, i-64] = +1.
    protT = np.zeros((128, 128), np.float32)
    for i in range(64):
        protT[i + 64, i] = -1.0
        protT[i, i + 64] = 1.0
    protT = protT.astype(bf)

    ident = np.eye(128, dtype=np.float32).astype(bf)

    # additive causal masks for the 4 diagonal [128k, 512q] tiles
    # valid iff q_local >= d*128 + k_local
    kl = np.arange(128)[:, None]
    ql = np.arange(512)[None, :]
    cmask = np.stack(
        [np.where(ql >= d * 128 + kl, 0.0, -1e9).astype(np.float32)
         for d in range(4)], axis=1)                   # [128, 4, 512]
    cmask = np.ascontiguousarray(cmask)

    xT = []
    for b in range(B):
        xT.append(np.ascontiguousarray(x[b].astype(bf).T))

    wqkv, wob = [], []
    for g in range(4):
        q_s = (wq[:, g * 1024:(g + 1) * 1024] * scale).astype(bf)
        k_s = wk[:, g * 256:(g + 1) * 256].astype(bf)
        v_s = wv[:, g * 256:(g + 1) * 256].astype(bf)
        wall = np.concatenate([q_s, k_s, v_s], axis=1)       # [C, 1536]
        wall = wall.reshape(CCH, 128, 2, 768).transpose(0, 2, 1, 3)
        wqkv.append(np.ascontiguousarray(wall))              # [32, 2, 128, 768]
        wob.append(np.ascontiguousarray(
            wo[g * 1024:(g + 1) * 1024, :].astype(bf)))      # [1024, C]

    in_maps = []
    for core in range(NCORES):
        b, g = core // 4, core % 4
        in_maps.append({
            "xT": xT[b], "wqkv": wqkv[g], "wo": wob[g],
            "cosT": cosT, "sinT": sinT, "protT": protT,
            "ident": ident, "cmask": cmask,
        })
    return in_maps


def kernel(x, wq, wk, wv, wo, _trace=False, _tmpdir=None):
    if "nc" not in _CACHE:
        _CACHE["nc"] = _build_nc()
    nc = _CACHE["nc"]

    in_maps = _host_prep(x, wq, wk, wv, wo)
    res = run_bass_kernel_spmd(nc, in_maps, core_ids=list(range(NCORES)),
                               trace=_trace, tmpdir=_tmpdir)
    _CACHE["last_results"] = res

    out = np.zeros((B, T, C), np.float32)
    for core in range(NCORES):
        out[core // 4] += res.results[core]["out"]
    return out
